# revision 69
# baseline (speedup 1.0000x reference)
"""Trainium2 Bass kernel for nn_AgentPolicy (sparse attention agent policy).

Data parallel over 8 NeuronCores: batch 4096 -> 512 per core.

Math (per batch row b; only comm[:, 0] of the reference attention is used):
  x  = relu(relu(obs@W1+b1)@W2+b2)                       [256]
  tok = [x, msg_0..62]                                   [64, 256]
  q0 = x@Wq + bq ; u = Wk q0          (bk drops: softmax/top-k shift-invariant)
  s_t = (u . tok_t)/16                                   scores row 0
  tokb_m = block sums of tok ; qb0 = (tokb0/16)@Wq+bq ; ub = Wk qb0
  sblk_m = (ub . tokb_m)/16 ; keep top-2 blocks (f32 selection path)
  per-block: mx_m, prob = exp(s - mx_m), Zh_m = sum prob
  E_m = sum_t prob_t tok_t   (ACT fp16 products + DVE in-place fp16 trees)
  flash merge: M = max mx_m ; f_m = keep_m exp(mx_m - M) ; Z = sum f_m Zh_m
  ctx = (sum f_m E_m)/Z
  out = relu((x + ctx@(Wv@Wo) + bv@Wo + bo)@W3 + b3)@W4 + b4

Engine split per 128-row tile (cost-model busy): DVE scores/sblk/fp16 E-trees
~34us, Pool f32 pair-sum tree via TensorTensor adds ~33us (only Pool-legal f32
arith), ACT fp16 products/exp/PSUM copies ~34us, DMA ~24.7us. Software
pipeline L(k) F(k) B(k-2) M2(k-1) M1(k): exps/products lag their scores by a
stage and the fp16 E-trees lag products by a stage, so every cross-engine
dependency is satisfied a stage ahead of its in-order consumer.
"""
import sys
import numpy as np

sys.path.insert(0, '/opt/trn_rl_repo')

import concourse.bass as bass
import concourse.mybir as mybir
import concourse.tile as tile
from concourse import bacc
from concourse.bass_utils import run_bass_kernel_spmd
from concourse.masks import make_identity

F32 = mybir.dt.float32
F32R = mybir.dt.float32r
F16 = mybir.dt.float16
AL = mybir.AluOpType
AF = mybir.ActivationFunctionType
AX = mybir.AxisListType

N_CORES = 8
B = 4096
B_CORE = B // N_CORES          # 512
P = 128                        # partition / b-tile size
N_TILES = B_CORE // P          # 4
OBS = 512
D = 256
HID = 128
NMSG = 63
S = 64                         # tokens = 1 + NMSG
NB = 4                         # blocks
BLK = 16
SCALE = 1.0 / 16.0             # 1/sqrt(D) == 1/BLK


def build_kernel(reps=1):
    nc = bacc.Bacc(None, target_bir_lowering=False)

    obs_d = nc.declare_dram_parameter("local_obs", [B_CORE, OBS], F32, isOutput=False)
    msg_d = nc.declare_dram_parameter("messages", [B_CORE, NMSG, D], F32, isOutput=False)
    W1_d = nc.declare_dram_parameter("W1", [OBS, HID], F32R, isOutput=False)
    b1_d = nc.declare_dram_parameter("b1", [HID], F32, isOutput=False)
    W2_d = nc.declare_dram_parameter("W2", [HID, D], F32R, isOutput=False)
    b2_d = nc.declare_dram_parameter("b2", [D], F32, isOutput=False)
    W3_d = nc.declare_dram_parameter("W3", [D, HID], F32R, isOutput=False)
    b3_d = nc.declare_dram_parameter("b3", [HID], F32, isOutput=False)
    W4_d = nc.declare_dram_parameter("W4", [HID, D], F32R, isOutput=False)
    b4_d = nc.declare_dram_parameter("b4", [D], F32, isOutput=False)
    Wq_d = nc.declare_dram_parameter("Wq", [D, D], F32R, isOutput=False)
    bq_d = nc.declare_dram_parameter("bq", [D], F32, isOutput=False)
    Wk_d = nc.declare_dram_parameter("Wk", [D, D], F32R, isOutput=False)
    bk_d = nc.declare_dram_parameter("bk", [D], F32, isOutput=False)  # unused (invariance)
    Wv_d = nc.declare_dram_parameter("Wv", [D, D], F32R, isOutput=False)
    bv_d = nc.declare_dram_parameter("bv", [D], F32R, isOutput=False)
    Wo_d = nc.declare_dram_parameter("Wo", [D, D], F32R, isOutput=False)
    bo_d = nc.declare_dram_parameter("bo", [D], F32, isOutput=False)
    out_d = nc.declare_dram_parameter("out", [B_CORE, D], F32, isOutput=True)

    with tile.TileContext(nc) as tc:
        with tc.tile_pool(name="stat", bufs=1) as stat, \
             tc.tile_pool(name="work", bufs=2) as work, \
             tc.tile_pool(name="w3", bufs=3) as w3, \
             tc.tile_pool(name="one", bufs=1) as one, \
             tc.tile_pool(name="loc", bufs=1) as loc, \
             tc.tile_pool(name="psA", bufs=2, space="PSUM") as psA, \
             tc.tile_pool(name="psB", bufs=4, space="PSUM") as psB:

            # ---------------- static setup ----------------
            ident = stat.tile([P, P], F32)
            make_identity(nc, ident[:])

            W1s = stat.tile([P, 4, HID], F32R)
            nc.sync.dma_start(out=W1s[:], in_=W1_d.rearrange("(a p) m -> p a m", p=P))
            W2s = stat.tile([P, 1, D], F32R)
            nc.sync.dma_start(out=W2s[:], in_=W2_d.rearrange("(a p) m -> p a m", p=P))
            Wqs = stat.tile([P, 2, D], F32R)
            nc.sync.dma_start(out=Wqs[:], in_=Wq_d.rearrange("(a p) m -> p a m", p=P))
            Wks = stat.tile([P, 2, D], F32R)
            nc.sync.dma_start(out=Wks[:], in_=Wk_d.rearrange("(a p) m -> p a m", p=P))
            b1s = stat.tile([P, 1], F32)
            nc.sync.dma_start(out=b1s[:], in_=b1_d.rearrange("(a p) -> p a", p=P))
            b2s = stat.tile([P, 2], F32)
            nc.sync.dma_start(out=b2s[:], in_=b2_d.rearrange("(a p) -> p a", p=P))
            bqs = stat.tile([P, 2], F32)
            nc.sync.dma_start(out=bqs[:], in_=bq_d.rearrange("(a p) -> p a", p=P))
            WkT = stat.tile([P, 2, D], F32R)
            for it in range(2):
                for nt in range(2):
                    pt = psA.tile([P, P], F32, tag="tpA")
                    nc.tensor.transpose(pt[:], Wks[:, it, nt * P:(nt + 1) * P].bitcast(F32), ident[:])
                    nc.scalar.activation(WkT[:, nt, it * P:(it + 1) * P], pt[:], AF.Identity)

            def setup_late():
                W3s_l = stat.tile([P, 2, HID], F32R, name="W3s")
                nc.sync.dma_start(out=W3s_l[:], in_=W3_d.rearrange("(a p) m -> p a m", p=P))
                W4s_l = stat.tile([P, 1, D], F32R, name="W4s")
                nc.sync.dma_start(out=W4s_l[:], in_=W4_d.rearrange("(a p) m -> p a m", p=P))
                Wvs_l = stat.tile([P, 2, D], F32R, name="Wvs")
                nc.sync.dma_start(out=Wvs_l[:], in_=Wv_d.rearrange("(a p) m -> p a m", p=P))
                Wos_l = stat.tile([P, 2, D], F32R, name="Wos")
                nc.sync.dma_start(out=Wos_l[:], in_=Wo_d.rearrange("(a p) m -> p a m", p=P))
                b3s_l = stat.tile([P, 1], F32, name="b3s")
                nc.sync.dma_start(out=b3s_l[:], in_=b3_d.rearrange("(a p) -> p a", p=P))
                b4row_l = stat.tile([1, D], F32, name="b4row")
                nc.sync.dma_start(out=b4row_l[:], in_=b4_d.rearrange("(a d) -> a d", a=1))
                ones1_l = stat.tile([1, P], F32, name="ones1")
                nc.vector.memset(ones1_l[:], 1.0)
                bvs_l = stat.tile([P, 2], F32R, name="bvs")
                nc.sync.dma_start(out=bvs_l[:], in_=bv_d.rearrange("(a p) -> p a", p=P))
                bos_l = stat.tile([P, 2], F32, name="bos")
                nc.sync.dma_start(out=bos_l[:], in_=bo_d.rearrange("(a p) -> p a", p=P))
                WvT_l = stat.tile([P, 2, D], F32R, name="WvT")
                for it in range(2):
                    for nt in range(2):
                        pt = psA.tile([P, P], F32, tag="tpA")
                        nc.tensor.transpose(pt[:], Wvs_l[:, it, nt * P:(nt + 1) * P].bitcast(F32), ident[:])
                        nc.scalar.activation(WvT_l[:, nt, it * P:(it + 1) * P], pt[:], AF.Identity)
                Wvo_l = stat.tile([P, 2, D], F32R, name="Wvo")
                for it in range(2):
                    pt = psA.tile([P, D], F32, tag="peu")
                    for nt in range(2):
                        nc.tensor.matmul(pt[:], WvT_l[:, nt, it * P:(it + 1) * P],
                                         Wos_l[:, nt, :], start=(nt == 0), stop=(nt == 1))
                    nc.scalar.activation(Wvo_l[:, it, :], pt[:], AF.Identity)
                bvo_l = stat.tile([P, 2], F32, name="bvo")
                for ot in range(2):
                    pt = psA.tile([P, 1], F32, tag="tpA")
                    for nt in range(2):
                        nc.tensor.matmul(pt[:], Wos_l[:, nt, ot * P:(ot + 1) * P].bitcast(F32),
                                         bvs_l[:, nt:nt + 1].bitcast(F32), start=(nt == 0), stop=(nt == 1))
                    nc.scalar.activation(bvo_l[:, ot:ot + 1], pt[:], AF.Identity)
                nc.vector.tensor_tensor(bvo_l[:], bvo_l[:], bos_l[:], op=AL.add)
                return W3s_l, W4s_l, b3s_l, b4row_l, ones1_l, Wvo_l, bvo_l

            # shared single-buffer scratch
            s32 = one.tile([P, 4, D], F32, name="s32")         # pair sums, reused per quarter
            qs = one.tile([P, 2, D], F32, name="qs")           # per-quarter sums
            tokb = one.tile([P, NB, D], F32, name="tokb")
            ub_b = one.tile([P, D], F32, name="ub_b")
            dmD = one.tile([P, 1], F32, name="dmD")            # dummy AMR out
            mxs = one.tile([P, 1], F32, name="mxs")
            thr = one.tile([P, 1], F32, name="thr")
            minn = one.tile([P, 1], F32, name="minn")
            Zs = one.tile([P, 1], F32, name="Zs")
            rZ = one.tile([P, 1], F32, name="rZ")
            iseq = one.tile([P, NB], F32, name="iseq")
            masked = one.tile([P, NB], F32, name="masked")

            state = {}

            # ---------------- per-tile stages ----------------
            def load(k, b0):
                st = state[k] = {}
                obs_sb = work.tile([P, OBS], F32, tag="obs", bufs=1)
                st["obs"] = obs_sb
                nc.sync.dma_start(out=obs_sb[:], in_=obs_d[b0:b0 + P, :])
                tok = work.tile([P, S, D], F32, tag="tok")
                st["tok"] = tok
                # per-block chunks: each score/tree block unblocks as its chunk lands
                for m in range(NB):
                    t0 = max(1, m * BLK)
                    t1 = (m + 1) * BLK
                    nc.sync.dma_start(out=tok[:, t0:t1, :],
                                      in_=msg_d[b0:b0 + P, t0 - 1:t1 - 1, :])

            def front(k):
                st = state[k]
                obs_sb, tok = st["obs"], st["tok"]

                # encoder
                obs_T = loc.tile([P, 4, P], F32R, tag="obsT")
                for kt in range(4):
                    pt = psA.tile([P, P], F32, tag="tpA")
                    nc.tensor.transpose(pt[:], obs_sb[:, kt * P:(kt + 1) * P], ident[:])
                    nc.scalar.activation(obs_T[:, kt, :], pt[:], AF.Identity)
                x1_T = loc.tile([P, P], F32R, tag="x1hT")
                pe1 = psB.tile([P, P], F32, tag="pe")
                for kt in range(4):
                    nc.tensor.matmul(pe1[:], W1s[:, kt, :], obs_T[:, kt, :],
                                     start=(kt == 0), stop=(kt == 3))
                nc.scalar.activation(x1_T[:], pe1[:], AF.Relu, bias=b1s[:])
                x_T = w3.tile([P, 2, P], F32R, tag="xT")
                st["x_T"] = x_T
                for ot in range(2):
                    pe = psB.tile([P, P], F32, tag="pe")
                    nc.tensor.matmul(pe[:], W2s[:, 0, ot * P:(ot + 1) * P], x1_T[:],
                                     start=True, stop=True)
                    nc.scalar.activation(x_T[:, ot, :], pe[:], AF.Relu, bias=b2s[:, ot:ot + 1])
                # x -> tok[:, 0, :]
                for ot in range(2):
                    pt = psA.tile([P, P], F32, tag="tpA")
                    nc.tensor.transpose(pt[:], x_T[:, ot, :].bitcast(F32), ident[:])
                    nc.scalar.activation(tok[:, 0, ot * P:(ot + 1) * P], pt[:], AF.Identity)

                # q0_T = Wq^T x_T + bq ; u = Wk q0 (b-major)
                q0_T = loc.tile([P, 2, P], F32R, tag="qT")
                for ot in range(2):
                    pe = psB.tile([P, P], F32, tag="pe")
                    for kt in range(2):
                        nc.tensor.matmul(pe[:], Wqs[:, kt, ot * P:(ot + 1) * P], x_T[:, kt, :],
                                         start=(kt == 0), stop=(kt == 1))
                    nc.scalar.activation(q0_T[:, ot, :], pe[:], AF.Identity, bias=bqs[:, ot:ot + 1])
                u_b = work.tile([P, D], F32, tag="ub")
                st["u_b"] = u_b
                peu = psA.tile([P, D], F32, tag="peu")
                for kt in range(2):
                    nc.tensor.matmul(peu[:], q0_T[:, kt, :], WkT[:, kt, :],
                                     start=(kt == 0), stop=(kt == 1))
                nc.scalar.activation(u_b[:], peu[:], AF.Identity)

            def tree_block(tok, m):
                # Pool pair-sum tree for block m, one 8-token quarter at a time
                # (TensorTensor adds only: the only Pool-legal f32 arith)
                for q in range(2):
                    t0 = BLK * m + 8 * q
                    nc.gpsimd.tensor_tensor(
                        s32[:], tok[:, t0:t0 + 8:2, :], tok[:, t0 + 1:t0 + 8:2, :], op=AL.add)
                    nc.gpsimd.tensor_tensor(
                        s32[:, 0:4:2, :], s32[:, 0:4:2, :], s32[:, 1:4:2, :], op=AL.add)
                    nc.gpsimd.tensor_tensor(
                        qs[:, q, :], s32[:, 0, :], s32[:, 2, :], op=AL.add)
                nc.gpsimd.tensor_tensor(
                    tokb[:, m, :], qs[:, 0, :], qs[:, 1, :], op=AL.add)

            def trees_block(k, m):
                # deferred DVE fp16 in-place tree over tile k's block-m products;
                # E_m ends up aliased at prods[2m][:, 0, :]
                st = state[k]
                ph = [st["prods"][m * 2 + h] for h in range(2)]
                for p_ in ph:
                    nc.vector.tensor_tensor(p_[:, 0:8:2, :], p_[:, 0:8:2, :],
                                            p_[:, 1:8:2, :], op=AL.add)
                    nc.vector.tensor_tensor(p_[:, 0:8:4, :], p_[:, 0:8:4, :],
                                            p_[:, 2:8:4, :], op=AL.add)
                    nc.vector.tensor_tensor(p_[:, 0, :], p_[:, 0, :], p_[:, 4, :], op=AL.add)
                nc.vector.tensor_tensor(ph[0][:, 0, :], ph[0][:, 0, :], ph[1][:, 0, :], op=AL.add)

            def trees(k):
                st = state[k]
                for m in st["treeblocks"]:
                    ph = [st["prods"][m * 2 + h] for h in range(2)]
                    for p_ in ph:
                        nc.vector.tensor_tensor(p_[:, 0:8:2, :], p_[:, 0:8:2, :],
                                                p_[:, 1:8:2, :], op=AL.add)
                        nc.vector.tensor_tensor(p_[:, 0:8:4, :], p_[:, 0:8:4, :],
                                                p_[:, 2:8:4, :], op=AL.add)
                        nc.vector.tensor_tensor(p_[:, 0, :], p_[:, 0, :], p_[:, 4, :], op=AL.add)
                    nc.vector.tensor_tensor(ph[0][:, 0, :], ph[0][:, 0, :], ph[1][:, 0, :], op=AL.add)

            def mid1(k):
                st = state[k]
                tok, u_b = st["tok"], st["u_b"]

                sc = work.tile([P, S], F32, tag="sc")
                st["sc"] = sc
                nmx = work.tile([P, NB], F32, tag="nmx")
                st["nmx"] = nmx
                prob = work.tile([P, S], F32, tag="prob")
                st["prob"] = prob
                Zh = work.tile([P, NB], F32, tag="Zh")
                st["Zh"] = Zh
                sblk = work.tile([P, NB], F32, tag="sblk")

                def exp_block(m):
                    nc.scalar.activation(
                        prob[:, m * BLK:(m + 1) * BLK], sc[:, m * BLK:(m + 1) * BLK],
                        AF.Exp, bias=nmx[:, m:m + 1], accum_out=Zh[:, m:m + 1])
                st["exp_block"] = exp_block

                def score_block(m):
                    for j in range(BLK):
                        t = m * BLK + j
                        nc.vector.affine_mul_reduce(
                            out=dmD.broadcast_to((P, D)), accum_out=sc[:, t:t + 1],
                            in0=tok[:, t, :], in1=u_b[:], scale=SCALE, bias=0.0)
                    nc.vector.tensor_reduce(
                        out=nmx[:, m:m + 1], in_=sc[:, m * BLK:(m + 1) * BLK],
                        axis=AX.X, op=AL.max, negate=True)

                def sblk_dot(m):
                    nc.vector.affine_mul_reduce(
                        out=dmD.broadcast_to((P, D)), accum_out=sblk[:, m:m + 1],
                        in0=tokb[:, m, :], in1=ub_b[:], scale=SCALE, bias=0.0)

                # Pool trees; DVE scores run in parallel
                for m in range(NB):
                    tree_block(tok, m)
                st["fprods"] = []
                for m in range(NB):
                    score_block(m)
                    if k == 0 or k == n - 1:
                        exp_block(m)
                    if k >= 1:
                        trees_block(k - 1, m)
                    if k == n - 1:
                        # fused products: slots freed by trees_block(k-1, m) above
                        for h in range(2):
                            prods = work.tile([P, 8, D], F16, tag="prods", bufs=8)
                            st["fprods"].append(prods)
                            for j in range(8):
                                t = m * BLK + h * 8 + j
                                nc.scalar.activation(prods[:, j, :], tok[:, t, :], AF.Copy,
                                                     scale=prob[:, t:t + 1])
                # flash-merge scalars: M = max mx_m = -min nmx ; em = exp(-nmx+minn)
                em = work.tile([P, NB], F32, tag="em")
                st["em"] = em
                nc.vector.tensor_reduce(out=minn[:], in_=nmx[:], axis=AX.X, op=AL.min)
                nc.scalar.activation(em[:], nmx[:], AF.Exp, scale=-1.0, bias=minn[:])

                # qb0 = (tokb0/16)@Wq + bq ; ub = Wk qb0 (b-major, f32)
                tokb0_T = loc.tile([P, 2, P], F32R, tag="t0yT")
                for ot in range(2):
                    pt = psA.tile([P, P], F32, tag="tpA")
                    nc.tensor.transpose(pt[:], tokb[:, 0, ot * P:(ot + 1) * P], ident[:])
                    nc.scalar.activation(tokb0_T[:, ot, :], pt[:], AF.Identity)
                qb0_T = loc.tile([P, 2, P], F32R, tag="qT")
                for ot in range(2):
                    pe = psB.tile([P, P], F32, tag="pe")
                    for kt in range(2):
                        nc.tensor.matmul(pe[:], Wqs[:, kt, ot * P:(ot + 1) * P], tokb0_T[:, kt, :],
                                         start=(kt == 0), stop=(kt == 1))
                    nc.scalar.activation(qb0_T[:, ot, :], pe[:], AF.Identity,
                                         scale=SCALE, bias=bqs[:, ot:ot + 1])
                peub = psA.tile([P, D], F32, tag="peu")
                for kt in range(2):
                    nc.tensor.matmul(peub[:], qb0_T[:, kt, :], WkT[:, kt, :],
                                     start=(kt == 0), stop=(kt == 1))
                nc.scalar.activation(ub_b[:], peub[:], AF.Identity)
                sblk_dot(0)
                sblk_dot(1)
                sblk_dot(2)
                sblk_dot(3)

                # top-2 keep mask (f32, all DVE-local)
                keep = work.tile([P, NB], F32, tag="keep")
                st["keep"] = keep
                nc.vector.tensor_reduce(out=mxs[:], in_=sblk[:], axis=AX.X, op=AL.max)
                nc.vector.tensor_tensor(iseq[:], sblk[:], mxs[:].broadcast_to((P, NB)), op=AL.is_ge)
                nc.vector.scalar_tensor_tensor(
                    out=masked[:], in0=iseq[:], scalar=-3e38, in1=sblk[:],
                    op0=AL.mult, op1=AL.add)
                nc.vector.tensor_reduce(out=thr[:], in_=masked[:], axis=AX.X, op=AL.max)
                nc.vector.tensor_tensor(keep[:], sblk[:], thr[:].broadcast_to((P, NB)), op=AL.is_ge)

            def mid2(k, hybrid=False):
                # ACT exps + fp16 products + (last tile) DVE chains
                st = state[k]
                tok, prob = st["tok"], st["prob"]
                if 0 < k < n - 1:
                    for m in range(NB):
                        st["exp_block"](m)
                if hybrid:
                    # last tile: products were fused into mid1's score loop
                    pr = st["fprods"]
                    st["prods"] = pr
                    st["treeblocks"] = tuple(range(NB))
                    st["Eaps"] = [pr[2 * m][:, 0, :] for m in range(NB)]
                    return
                chainblocks = ()
                st["treeblocks"] = tuple(m for m in range(NB) if m not in chainblocks)
                pr = []
                for m in range(NB):
                    for h in range(2):
                        prods = work.tile([P, 8, D], F16, tag="prods", bufs=8)
                        pr.append(prods)
                        if m in chainblocks:
                            continue
                        for j in range(8):
                            t = m * BLK + h * 8 + j
                            nc.scalar.activation(prods[:, j, :], tok[:, t, :], AF.Copy,
                                                 scale=prob[:, t:t + 1])
                st["prods"] = pr
                for m in chainblocks:
                    acc = pr[2 * m][:, 0, :]
                    t0 = m * BLK
                    nc.vector.tensor_scalar(acc, tok[:, t0, :], prob[:, t0:t0 + 1],
                                            None, op0=AL.mult)
                    for t in range(t0 + 1, t0 + BLK):
                        nc.vector.scalar_tensor_tensor(
                            out=acc, in0=tok[:, t, :], scalar=prob[:, t:t + 1],
                            in1=acc, op0=AL.mult, op1=AL.add)
                st["Eaps"] = [pr[2 * m][:, 0, :] for m in range(NB)]

            def back(k, b0):
                st = state.pop(k)
                x_T, nmx, Zh, keep, Eaps = st["x_T"], st["nmx"], st["Zh"], st["keep"], st["Eaps"]

                # flash merge: f = keep * em (em precomputed in mid1)
                em = st["em"]
                f = work.tile([P, NB], F32, tag="f")
                nc.vector.tensor_tensor(f[:], em[:], keep[:], op=AL.mult)
                nc.vector.affine_mul_reduce(
                    out=dmD.broadcast_to((P, NB)), accum_out=Zs[:],
                    in0=Zh[:], in1=f[:], scale=1.0, bias=0.0)
                nc.vector.reciprocal(rZ[:], Zs[:])
                ctx3 = loc.tile([P, 2, P], F32, tag="qT")
                ctx = ctx3[:].rearrange("p a b -> p (a b)")
                nc.vector.tensor_scalar(ctx, Eaps[0], f[:, 0:1], None, op0=AL.mult)
                for m in range(1, NB):
                    nc.vector.scalar_tensor_tensor(
                        out=ctx, in0=Eaps[m], scalar=f[:, m:m + 1],
                        in1=ctx, op0=AL.mult, op1=AL.add)
                nc.vector.tensor_scalar(ctx, ctx, rZ[:], None, op0=AL.mult)

                # decode: y_T = Wvo^T ctx_T + x_T (PE identity seed) + bvo
                ctx_T = loc.tile([P, 2, P], F32R, tag="ctxT")
                for ot in range(2):
                    pt = psA.tile([P, P], F32, tag="tpA")
                    nc.tensor.transpose(pt[:], ctx3[:, ot, :], ident[:])
                    nc.scalar.activation(ctx_T[:, ot, :], pt[:], AF.Identity)
                y_T = loc.tile([P, 2, P], F32R, tag="t0yT")
                for ot in range(2):
                    pe = psB.tile([P, P], F32, tag="pe")
                    for kt in range(2):
                        nc.tensor.matmul(pe[:], Wvo[:, kt, ot * P:(ot + 1) * P],
                                         ctx_T[:, kt, :], start=(kt == 0), stop=False)
                    nc.tensor.matmul(pe[:], ident[:], x_T[:, ot, :].bitcast(F32),
                                     start=False, stop=True)
                    nc.scalar.activation(y_T[:, ot, :], pe[:], AF.Identity, bias=bvo[:, ot:ot + 1])
                h_T = loc.tile([P, P], F32R, tag="hT")
                ped = psB.tile([P, P], F32, tag="pe")
                for kt in range(2):
                    nc.tensor.matmul(ped[:], W3s[:, kt, :], y_T[:, kt, :],
                                     start=(kt == 0), stop=(kt == 1))
                nc.scalar.activation(h_T[:], ped[:], AF.Relu, bias=b3s[:])
                out_b = loc.tile([P, D], F32, tag="outb")
                peo = psA.tile([P, D], F32, tag="peu")
                nc.tensor.matmul(peo[:], ones1[:], b4row[:], start=True, stop=False)
                nc.tensor.matmul(peo[:], h_T[:], W4s[:, 0, :], start=False, stop=True)
                nc.scalar.activation(out_b[:], peo[:], AF.Copy)
                nc.sync.dma_start(out=out_d[b0:b0 + P, :], in_=out_b[:])

            # ---------------- software-pipelined emission ----------------
            tiles = [t for _ in range(reps) for t in range(N_TILES)]
            n = len(tiles)
            W3s, W4s, b3s, b4row, ones1, Wvo, bvo = setup_late()
            order = []
            for k in range(n):
                order.append(("L", k))
                order.append(("F", k))
                if k >= 2:
                    order.append(("B", k - 2))
                if k >= 1:
                    order.append(("M2", k - 1))
                order.append(("M1", k))
            order += [("M2", n - 1), ("B", n - 2), ("T", n - 1), ("B", n - 1)]
            for kind, k in order:
                if kind == "L":
                    load(k, tiles[k] * P)
                elif kind == "F":
                    front(k)
                elif kind == "M1":
                    mid1(k)
                elif kind == "M2":
                    mid2(k, hybrid=(k == n - 1))
                elif kind == "T":
                    trees(k)
                else:
                    back(k, tiles[k] * P)

    nc.compile()
    return nc


_NC_CACHE = None


def kernel(**inputs):
    global _NC_CACHE
    if _NC_CACHE is None:
        _NC_CACHE = build_kernel()
    nc = _NC_CACHE

    weights = {k: np.ascontiguousarray(v, dtype=np.float32) for k, v in inputs.items()
               if k not in ("local_obs", "messages")}
    obs = np.ascontiguousarray(inputs["local_obs"], dtype=np.float32)
    msg = np.ascontiguousarray(inputs["messages"], dtype=np.float32)

    in_maps = []
    for c in range(N_CORES):
        m = dict(weights)
        m["local_obs"] = obs[c * B_CORE:(c + 1) * B_CORE]
        m["messages"] = msg[c * B_CORE:(c + 1) * B_CORE]
        in_maps.append(m)

    res = run_bass_kernel_spmd(nc, in_maps, list(range(N_CORES))).results
    return np.concatenate([r["out"] for r in res], axis=0)
